# revision 4
# baseline (speedup 1.0000x reference)
"""GCN layer (message passing + linear) on 8 Trainium2 NeuronCores.

Strategy (per sharding hint): nodes are sharded across the 8 cores
(12544 = 98 blocks of 128 rows each); edges are partitioned by target
node so the scatter-add stays local to the target's shard. The full
(padded) x is staged in every core's DRAM, so "all-gather source
features" is an indirect-DMA row gather from local HBM.

Per 128-node block, the aggregation sum_{e: tgt in block} norm_e * x[src_e]
is computed by the TensorEngine: for each 128-edge tile, a selection
matrix S[e, j] = norm_e * (tgt_e == block_base + j) is built on the
Vector engine (one fused tensor_scalar: is_equal against a constant
iota, then scaled by the per-edge norm), and S^T @ msgs accumulates in
PSUM over the block's tiles.  Finalize: + x residual, PE transpose,
@ W^T with bias folded in as a rank-1 matmul, DMA out.

Host-side work is restricted to integer graph preprocessing (sort by
target, slot layout, degree counts and the scalar norm coefficients
derived from them).  All O(E*D) / O(N*D) tensor math runs on-device.
"""

import os
import sys

import numpy as np

for _p in ("/opt/trn_rl_repo", "/root/.axon_site/_ro/trn_rl_repo"):
    if os.path.isdir(_p) and _p not in sys.path:
        sys.path.append(_p)

from concourse import bacc, bass, mybir
import concourse.tile as tile
from concourse.masks import make_identity

P = 128            # SBUF partitions / edge-tile size / node-block size
D = 64             # feature dim (in == out)
N_CORES = 8

F32 = mybir.dt.float32
BF16 = mybir.dt.bfloat16
I32 = mybir.dt.int32


def build_program(nb: int, k: int, xg_rows: int, msg_bf16: bool = True):
    """Build + compile the per-core SPMD program.

    nb: node blocks per core; k: edge tiles per block; xg_rows: rows of
    the (padded, replicated) gather source.
    """
    sh_rows = nb * P
    t_total = nb * k
    nc = bacc.Bacc("TRN2", target_bir_lowering=False, debug=False)

    xg = nc.dram_tensor("xg", [xg_rows, D], F32, kind="ExternalInput")
    xres = nc.dram_tensor("xres", [sh_rows, D], F32, kind="ExternalInput")
    offs = nc.dram_tensor("offs", [P, t_total], I32, kind="ExternalInput")
    trel = nc.dram_tensor("trel", [P, t_total], F32, kind="ExternalInput")
    norm = nc.dram_tensor("norm", [P, t_total], F32, kind="ExternalInput")
    wt = nc.dram_tensor("wt", [D, D], F32, kind="ExternalInput")
    bias = nc.dram_tensor("bias", [1, D], F32, kind="ExternalInput")
    out = nc.dram_tensor("out", [sh_rows, D], F32, kind="ExternalOutput")

    mdt = BF16 if msg_bf16 else F32

    with tile.TileContext(nc) as tc:
        with (
            tc.tile_pool(name="const", bufs=1) as cpool,
            tc.tile_pool(name="gath", bufs=4) as gpool,
            tc.tile_pool(name="msg", bufs=3) as mpool,
            tc.tile_pool(name="sel", bufs=6) as spool,
            tc.tile_pool(name="fin", bufs=3) as fpool,
            tc.tile_pool(name="ps_agg", bufs=2, space="PSUM") as ps_agg,
            tc.tile_pool(name="ps_t", bufs=2, space="PSUM") as ps_t,
            tc.tile_pool(name="ps_o", bufs=2, space="PSUM") as ps_o,
        ):
            iota_i = cpool.tile([P, P], I32)
            nc.gpsimd.iota(iota_i[:], pattern=[[1, P]], base=0, channel_multiplier=0)
            iota_f = cpool.tile([P, P], F32)
            nc.vector.tensor_copy(iota_f[:], iota_i[:])
            ident = cpool.tile([P, P], F32)
            make_identity(nc, ident[:])
            ones_row = cpool.tile([1, P], F32)
            nc.vector.memset(ones_row[:], 1.0)

            wt_sb = cpool.tile([D, D], F32)
            nc.sync.dma_start(wt_sb[:], wt[:])
            b_sb = cpool.tile([1, D], F32)
            nc.sync.dma_start(b_sb[:], bias[:])
            offs_sb = cpool.tile([P, t_total], I32)
            nc.sync.dma_start(offs_sb[:], offs[:])
            trel_sb = cpool.tile([P, t_total], F32)
            nc.sync.dma_start(trel_sb[:], trel[:])
            norm_sb = cpool.tile([P, t_total], F32)
            nc.sync.dma_start(norm_sb[:], norm[:])

            for b in range(nb):
                gath = gpool.tile([P, k * D], F32)
                # HW indirect DMA consumes one offset per partition per call.
                for kk in range(k):
                    nc.gpsimd.indirect_dma_start(
                        out=gath[:, kk * D : (kk + 1) * D],
                        out_offset=None,
                        in_=xg[:, :],
                        in_offset=bass.IndirectOffsetOnAxis(
                            ap=offs_sb[:, b * k + kk : b * k + kk + 1], axis=0
                        ),
                    )
                if msg_bf16:
                    msg = mpool.tile([P, k * D], BF16)
                    nc.scalar.copy(msg[:], gath[:])
                else:
                    msg = gath

                ps = ps_agg.tile([P, D], F32)
                for kk in range(k):
                    t = b * k + kk
                    sel = spool.tile([P, P], mdt)
                    nc.vector.tensor_scalar(
                        sel[:],
                        iota_f[:],
                        trel_sb[:, t : t + 1],
                        norm_sb[:, t : t + 1],
                        op0=mybir.AluOpType.is_equal,
                        op1=mybir.AluOpType.mult,
                    )
                    nc.tensor.matmul(
                        ps[:],
                        sel[:],
                        msg[:, kk * D : (kk + 1) * D],
                        start=(kk == 0),
                        stop=(kk == k - 1),
                    )

                xb = fpool.tile([P, D], F32, tag="xb")
                nc.sync.dma_start(xb[:], xres[b * P : (b + 1) * P, :])
                agg = fpool.tile([P, D], F32, tag="agg")
                nc.vector.tensor_add(agg[:], ps[:], xb[:])

                ps_tr = ps_t.tile([D, P], F32)
                nc.tensor.transpose(ps_tr[:], agg[:], ident[:])
                agg_t = fpool.tile([D, P], F32, tag="agg_t")
                nc.scalar.copy(agg_t[:], ps_tr[:])

                po = ps_o.tile([P, D], F32)
                nc.tensor.matmul(po[:], agg_t[:], wt_sb[:], start=True, stop=False)
                nc.tensor.matmul(po[:], ones_row[:], b_sb[:], start=False, stop=True)
                osb = fpool.tile([P, D], F32, tag="osb")
                nc.scalar.copy(osb[:], po[:])
                nc.sync.dma_start(out[b * P : (b + 1) * P, :], osb[:])

    nc.compile()
    return nc


def host_prep(x: np.ndarray, edge_index: np.ndarray, n_cores: int, nb: int):
    """Partition edges by target shard/block, build per-core slot arrays.

    Returns (in_maps, k, n_pad) where in_maps[c] feeds core c.
    """
    n, d = x.shape
    assert d == D
    total_blocks = n_cores * nb
    n_pad = total_blocks * P
    assert n_pad >= n

    src = np.ascontiguousarray(edge_index[0]).astype(np.int64)
    tgt = np.ascontiguousarray(edge_index[1]).astype(np.int64)
    e = src.shape[0]

    deg = np.bincount(tgt, minlength=n).astype(np.float32)
    dis = 1.0 / np.sqrt(np.maximum(deg, 1.0))
    norm_e = (dis[src] * dis[tgt]).astype(np.float32)

    order = np.argsort(tgt, kind="stable")
    src_s = src[order].astype(np.int32)
    tgt_s = tgt[order]
    norm_s = norm_e[order]

    blk = tgt_s >> 7  # // 128
    counts = np.bincount(blk, minlength=total_blocks)
    k = max(1, int(-(-counts.max() // P)))
    t_total = nb * k

    block_start = np.zeros(total_blocks + 1, np.int64)
    np.cumsum(counts, out=block_start[1:])
    rank = np.arange(e, dtype=np.int64) - block_start[blk]
    slot = blk * (k * P) + rank

    n_slots = total_blocks * k * P
    flat_off = np.zeros(n_slots, np.int32)
    flat_trel = np.full(n_slots, -1.0, np.float32)
    flat_norm = np.zeros(n_slots, np.float32)
    flat_off[slot] = src_s
    flat_trel[slot] = (tgt_s - (blk << 7)).astype(np.float32)
    flat_norm[slot] = norm_s

    xg = np.zeros((n_pad, D), np.float32)
    xg[:n] = x

    offs_all = flat_off.reshape(n_cores, t_total, P)
    trel_all = flat_trel.reshape(n_cores, t_total, P)
    norm_all = flat_norm.reshape(n_cores, t_total, P)

    in_maps = []
    for c in range(n_cores):
        in_maps.append(
            {
                "xg": xg,
                "xres": np.ascontiguousarray(xg[c * nb * P : (c + 1) * nb * P]),
                "offs": np.ascontiguousarray(offs_all[c].T),
                "trel": np.ascontiguousarray(trel_all[c].T),
                "norm": np.ascontiguousarray(norm_all[c].T),
            }
        )
    return in_maps, k, n_pad


_PROGRAM_CACHE: dict = {}


def kernel(x: np.ndarray, edge_index: np.ndarray, W: np.ndarray, b: np.ndarray) -> np.ndarray:
    from concourse.bass_utils import run_bass_kernel_spmd

    x = np.ascontiguousarray(x, dtype=np.float32)
    W = np.ascontiguousarray(W, dtype=np.float32)
    b = np.ascontiguousarray(b, dtype=np.float32)
    n = x.shape[0]

    rows_per_core = -(-n // N_CORES)
    nb = -(-rows_per_core // P)  # node blocks per core
    in_maps, k, n_pad = host_prep(x, edge_index, N_CORES, nb)
    wt = np.ascontiguousarray(W.T)
    brow = np.ascontiguousarray(b[None, :])
    for m in in_maps:
        m["wt"] = wt
        m["bias"] = brow

    key = (nb, k, n_pad)
    nc = _PROGRAM_CACHE.get(key)
    if nc is None:
        nc = build_program(nb, k, n_pad)
        _PROGRAM_CACHE[key] = nc

    res = run_bass_kernel_spmd(nc, in_maps, core_ids=list(range(N_CORES)))
    shards = [res.results[c]["out"] for c in range(N_CORES)]
    return np.concatenate(shards, axis=0)[:n].astype(np.float32)


# revision 12
# speedup vs baseline: 1.2566x; 1.2566x over previous
"""GCN layer (message passing + linear) on 8 Trainium2 NeuronCores.

Strategy (per sharding hint): nodes are sharded across the 8 cores
(12544 = 98 blocks of 128 rows each); edges are partitioned by target
node so the scatter-add stays local to the target's shard. The full
(padded) x is staged in every core's DRAM; "all-gather source features"
becomes a bulk `dma_gather` row gather from local HBM.

Per 128-node block, the aggregation sum_{e: tgt in block} norm_e * x[src_e]
runs on the TensorEngine: for each 128-edge tile, a selection matrix
S[e, j] = norm_e * (tgt_e == block_base + j) is built on the Vector
engine (one fused tensor_scalar: is_equal against a constant iota row,
times the per-edge norm), and S^T @ msgs accumulates in PSUM over the
block's tiles.  Finalize: + x residual, PE transpose, @ W^T with the
bias folded in as a rank-1 matmul, DMA out.

dma_gather details: indices are int16, so x is viewed as 4 row-chunks
of 25088 and each core's edges are sub-partitioned per (target block,
source chunk).  Per-site capacities are the max count over the 8 cores
(rounded to 128) so one SPMD program fits all cores; slack slots gather
row 0 of the chunk and are zeroed by the selection matrix (trel=-1).
Gather calls are batched per (super-block of SBS blocks, chunk) to
amortize the ~1us SWDGE fixed cost.

Host-side work is restricted to integer graph preprocessing (sorting,
slot layout, degree counts and the scalar norm coefficients derived
from them).  All O(E*D)/O(N*D) tensor math runs on-device.
"""

import os
import sys

import numpy as np

for _p in ("/opt/trn_rl_repo", "/root/.axon_site/_ro/trn_rl_repo"):
    if os.path.isdir(_p) and _p not in sys.path:
        sys.path.append(_p)

from concourse import bacc, bass, mybir
import concourse.tile as tile
from concourse.masks import make_identity

P = 128            # partitions / edge-tile size / node-block size
D = 64             # feature dim (in == out)
N_CORES = 8
N_CHUNKS = 4       # x row-chunks (int16 index limit: chunk rows < 32768)
SBS = 7            # blocks per gather super-block

F32 = mybir.dt.float32
BF16 = mybir.dt.bfloat16
I16 = mybir.dt.int16

_PROGRAM_CACHE: dict = {}


def make_layout(counts_max: np.ndarray, nb: int):
    """Static slot/tile layout shared by all cores.

    counts_max: [nb, N_CHUNKS] max-over-cores edge count per site.
    Returns dict with per-site capacities and tile bases.
    """
    G = -(-counts_max // P)  # [nb, nc] tiles per site
    kb = G.sum(axis=1)  # tiles per block
    n_sb = -(-nb // SBS)
    site_tile_base = np.zeros((nb, N_CHUNKS), np.int64)
    sb_tile_base = np.zeros(n_sb + 1, np.int64)
    t = 0
    for sb in range(n_sb):
        sb_tile_base[sb] = t
        blocks = range(sb * SBS, min((sb + 1) * SBS, nb))
        for c in range(N_CHUNKS):
            for b in blocks:
                site_tile_base[b, c] = t
                t += int(G[b, c])
        sb_tile_base[sb + 1] = t
    return {
        "G": G,
        "kb": kb,
        "nb": nb,
        "n_sb": n_sb,
        "site_tile_base": site_tile_base,
        "sb_tile_base": sb_tile_base,
        "T": int(t),
    }


def build_program(layout, n_pad: int, ch_rows: int, repeat: int = 1, sel_bf16: bool = False):
    nb = layout["nb"]
    G = layout["G"]
    n_sb = layout["n_sb"]
    site_tile_base = layout["site_tile_base"]
    sb_tile_base = layout["sb_tile_base"]
    T = layout["T"]
    sh_rows = nb * P

    nc = bacc.Bacc(
        "TRN2", target_bir_lowering=False, debug=False, num_swdge_queues=4
    )

    xg = nc.dram_tensor("xg", [n_pad, D], F32, kind="ExternalInput")
    xres = nc.dram_tensor("xres", [sh_rows, D], F32, kind="ExternalInput")
    gidx = nc.dram_tensor("gidx", [P, 8 * T], I16, kind="ExternalInput")
    trel = nc.dram_tensor("trel", [P, T], F32, kind="ExternalInput")
    norm = nc.dram_tensor("norm", [P, T], F32, kind="ExternalInput")
    wt = nc.dram_tensor("wt", [D, D], F32, kind="ExternalInput")
    bias = nc.dram_tensor("bias", [1, D], F32, kind="ExternalInput")
    out = nc.dram_tensor("out", [sh_rows, D], F32, kind="ExternalOutput")

    kb_max = int(layout["kb"].max()) if nb else 1
    w_sb_max = max(
        int(sb_tile_base[sb + 1] - sb_tile_base[sb]) for sb in range(n_sb)
    )

    with tile.TileContext(nc) as tc:
        with (
            tc.tile_pool(name="const", bufs=1) as cpool,
            tc.tile_pool(name="gath", bufs=2) as gpool,
            tc.tile_pool(name="msg", bufs=2) as mpool,
            tc.tile_pool(name="sel", bufs=6) as spool,
            tc.tile_pool(name="fin", bufs=3) as fpool,
            tc.tile_pool(name="ps_agg", bufs=4, space="PSUM") as ps_agg,
            tc.tile_pool(name="ps_t", bufs=2, space="PSUM") as ps_t,
            tc.tile_pool(name="ps_o", bufs=2, space="PSUM") as ps_o,
        ):
            iota_i = cpool.tile([P, P], mybir.dt.int32)
            nc.gpsimd.iota(iota_i[:], pattern=[[1, P]], base=0, channel_multiplier=0)
            iota_f = cpool.tile([P, P], F32)
            nc.vector.tensor_copy(iota_f[:], iota_i[:])
            ident = cpool.tile([P, P], F32)
            make_identity(nc, ident[:])
            ones_row = cpool.tile([1, P], F32)
            nc.vector.memset(ones_row[:], 1.0)

            wt_sb = cpool.tile([D, D], F32)
            nc.sync.dma_start(wt_sb[:], wt[:])
            b_sb = cpool.tile([1, D], F32)
            nc.sync.dma_start(b_sb[:], bias[:])
            gidx_sb = cpool.tile([P, 8 * T], I16)
            nc.sync.dma_start(gidx_sb[:], gidx[:])
            trel_f = cpool.tile([P, T], F32)
            nc.sync.dma_start(trel_f[:], trel[:])
            norm_f = cpool.tile([P, T], F32)
            nc.sync.dma_start(norm_f[:], norm[:])
            if sel_bf16:
                trel_sb = cpool.tile([P, T], BF16)
                nc.vector.tensor_copy(trel_sb[:], trel_f[:])
                norm_sb = cpool.tile([P, T], BF16)
                nc.vector.tensor_copy(norm_sb[:], norm_f[:])
                iota_sel = cpool.tile([P, P], BF16)
                nc.vector.tensor_copy(iota_sel[:], iota_f[:])
            else:
                trel_sb, norm_sb, iota_sel = trel_f, norm_f, iota_f

            for _rep in range(repeat):
              for sb in range(n_sb):
                blocks = list(range(sb * SBS, min((sb + 1) * SBS, nb)))
                t0 = int(sb_tile_base[sb])
                w_sb = int(sb_tile_base[sb + 1]) - t0
                gath = gpool.tile([P, w_sb_max, D], F32, tag="gath")
                # one gather call per source chunk, split to <= max_call_tiles
                max_call_tiles = 8
                for c in range(N_CHUNKS):
                    wc = int(sum(G[b, c] for b in blocks))
                    if wc == 0:
                        continue
                    ct0 = int(site_tile_base[blocks[0], c]) - t0
                    for o in range(0, wc, max_call_tiles):
                        w = min(max_call_tiles, wc - o)
                        n_idx = w * P
                        gcol = (t0 + ct0 + o) * 8
                        nc.gpsimd.dma_gather(
                            gath[:, ct0 + o : ct0 + o + w, :],
                            xg[c * ch_rows : (c + 1) * ch_rows, :],
                            gidx_sb[:, gcol : gcol + n_idx // 16],
                            n_idx,
                            n_idx,
                            D,
                            queue_num=c,
                        )
                msg = mpool.tile([P, w_sb_max, D], BF16, tag="msg")
                nc.scalar.copy(msg[:, :w_sb, :], gath[:, :w_sb, :])

                for b in blocks:
                    tiles = []
                    for c in range(N_CHUNKS):
                        for g in range(int(G[b, c])):
                            tiles.append(int(site_tile_base[b, c]) + g)
                    xb = fpool.tile([P, D], F32, tag="xb")
                    nc.sync.dma_start(xb[:], xres[b * P : (b + 1) * P, :])
                    agg = fpool.tile([P, D], F32, tag="agg")
                    if not tiles:
                        nc.vector.tensor_copy(agg[:], xb[:])
                    ps = ps_agg.tile([P, D], F32)
                    for i, t in enumerate(tiles):
                        sel = spool.tile([P, P], BF16)
                        nc.vector.tensor_scalar(
                            sel[:],
                            iota_sel[:],
                            trel_sb[:, t : t + 1],
                            norm_sb[:, t : t + 1],
                            op0=mybir.AluOpType.is_equal,
                            op1=mybir.AluOpType.mult,
                        )
                        nc.tensor.matmul(
                            ps[:],
                            sel[:],
                            msg[:, t - t0, :],
                            start=(i == 0),
                            stop=(i == len(tiles) - 1),
                        )

                    if tiles:
                        nc.vector.tensor_add(agg[:], ps[:], xb[:])

                    ps_tr = ps_t.tile([D, P], F32)
                    nc.tensor.transpose(ps_tr[:], agg[:], ident[:])
                    agg_t = fpool.tile([D, P], F32, tag="agg_t")
                    nc.scalar.copy(agg_t[:], ps_tr[:])

                    po = ps_o.tile([P, D], F32)
                    nc.tensor.matmul(po[:], agg_t[:], wt_sb[:], start=True, stop=False)
                    nc.tensor.matmul(
                        po[:], ones_row[:], b_sb[:], start=False, stop=True
                    )
                    osb = fpool.tile([P, D], F32, tag="osb")
                    nc.scalar.copy(osb[:], po[:])
                    nc.sync.dma_start(out[b * P : (b + 1) * P, :], osb[:])

    nc.compile()
    return nc


def host_prep(x: np.ndarray, edge_index: np.ndarray, n_cores: int, nb: int):
    """Partition edges by (target block, source chunk); build slot arrays.

    Returns (in_maps, layout, n_pad, ch_rows).
    """
    n, d = x.shape
    assert d == D
    total_blocks = n_cores * nb
    n_pad = total_blocks * P
    ch_rows = n_pad // N_CHUNKS
    assert n_pad % N_CHUNKS == 0 and ch_rows <= 32768

    src = np.ascontiguousarray(edge_index[0]).astype(np.int64)
    tgt = np.ascontiguousarray(edge_index[1]).astype(np.int64)
    e = src.shape[0]

    deg = np.bincount(tgt, minlength=n).astype(np.float32)
    dis = 1.0 / np.sqrt(np.maximum(deg, 1.0))
    norm_e = (dis[src] * dis[tgt]).astype(np.float32)

    chunk = src // ch_rows
    order = np.lexsort((chunk, tgt >> 7))
    src_s = src[order]
    tgt_s = tgt[order]
    norm_s = norm_e[order]
    chunk_s = chunk[order]

    blk = tgt_s >> 7  # global block id, 0..total_blocks-1
    seg_id = blk * N_CHUNKS + chunk_s
    seg_counts = np.bincount(seg_id, minlength=total_blocks * N_CHUNKS)
    seg_start = np.zeros(total_blocks * N_CHUNKS + 1, np.int64)
    np.cumsum(seg_counts, out=seg_start[1:])
    rank = np.arange(e, dtype=np.int64) - seg_start[seg_id]

    counts = seg_counts.reshape(n_cores, nb, N_CHUNKS)
    counts_max = counts.max(axis=0)  # [nb, N_CHUNKS]
    layout = make_layout(counts_max, nb)
    T = layout["T"]
    site_slot_base = layout["site_tile_base"] * P  # [nb, N_CHUNKS]

    core_of = blk // nb
    b_local = blk % nb
    slot = site_slot_base[b_local, chunk_s] + rank  # [e] core-local slot

    n_slots = T * P
    flat_trel = np.full((n_cores, n_slots), -1.0, np.float32)
    flat_norm = np.zeros((n_cores, n_slots), np.float32)
    flat_idx = np.zeros((n_cores, n_slots), np.int16)
    flat_trel[core_of, slot] = (tgt_s & 127).astype(np.float32)
    flat_norm[core_of, slot] = norm_s
    flat_idx[core_of, slot] = (src_s - chunk_s * ch_rows).astype(np.int16)

    xg = np.zeros((n_pad, D), np.float32)
    xg[:n] = x

    # wrap gather indices per call region: [sb][c] contiguous slot ranges
    sb_slot_base = layout["sb_tile_base"] * P
    G = layout["G"]
    n_sb = layout["n_sb"]
    gidx_all = np.empty((n_cores, P, 8 * T), np.int16)
    for sb in range(n_sb):
        blocks = range(sb * SBS, min((sb + 1) * SBS, nb))
        for c in range(N_CHUNKS):
            wc = int(sum(G[b, c] for b in blocks))
            if wc == 0:
                continue
            s0 = int(site_slot_base[blocks[0], c])
            L = wc * P
            seg = flat_idx[:, s0 : s0 + L]  # [cores, L]
            wrapped = seg.reshape(n_cores, L // 16, 16).transpose(0, 2, 1)
            gidx_all[:, :, s0 // 16 : (s0 + L) // 16] = np.tile(wrapped, (1, 8, 1))

    trel_all = flat_trel.reshape(n_cores, T, P).transpose(0, 2, 1)
    norm_all = flat_norm.reshape(n_cores, T, P).transpose(0, 2, 1)

    in_maps = []
    for c in range(n_cores):
        in_maps.append(
            {
                "xg": xg,
                "xres": np.ascontiguousarray(xg[c * nb * P : (c + 1) * nb * P]),
                "gidx": np.ascontiguousarray(gidx_all[c]),
                "trel": np.ascontiguousarray(trel_all[c]),
                "norm": np.ascontiguousarray(norm_all[c]),
            }
        )
    return in_maps, layout, n_pad, ch_rows


def kernel(x: np.ndarray, edge_index: np.ndarray, W: np.ndarray, b: np.ndarray) -> np.ndarray:
    from concourse.bass_utils import run_bass_kernel_spmd

    x = np.ascontiguousarray(x, dtype=np.float32)
    W = np.ascontiguousarray(W, dtype=np.float32)
    b = np.ascontiguousarray(b, dtype=np.float32)
    n = x.shape[0]

    rows_per_core = -(-n // N_CORES)
    nb = -(-rows_per_core // P)  # node blocks per core
    in_maps, layout, n_pad, ch_rows = host_prep(x, edge_index, N_CORES, nb)
    wt = np.ascontiguousarray(W.T)
    brow = np.ascontiguousarray(b[None, :])
    for m in in_maps:
        m["wt"] = wt
        m["bias"] = brow

    key = (nb, n_pad, layout["G"].tobytes())
    nc = _PROGRAM_CACHE.get(key)
    if nc is None:
        nc = build_program(layout, n_pad, ch_rows)
        _PROGRAM_CACHE[key] = nc

    res = run_bass_kernel_spmd(nc, in_maps, core_ids=list(range(N_CORES)))
    shards = [res.results[c]["out"] for c in range(N_CORES)]
    return np.concatenate(shards, axis=0)[:n].astype(np.float32)


# revision 16
# speedup vs baseline: 212.5665x; 169.1616x over previous
"""GCN layer (message passing + linear) on 8 Trainium2 NeuronCores.

Strategy (per sharding hint): nodes are sharded across the 8 cores
(12544 = 98 blocks of 128 rows each); edges are partitioned by target
node so the scatter-add stays local to the target's shard. The full
(padded) x is staged in every core's DRAM; "all-gather source features"
becomes a bulk `dma_gather` row gather from local HBM.

Per 128-node block, the aggregation sum_{e: tgt in block} norm_e * x[src_e]
runs on the TensorEngine: for each 128-edge tile, a selection matrix
S[e, j] = norm_e * (tgt_e == block_base + j) is built on the Vector
engine (one fused tensor_scalar: is_equal against a constant iota row,
times the per-edge norm), and S^T @ msgs accumulates in PSUM over the
block's tiles.  Finalize: + x residual, PE transpose, @ W^T with the
bias folded in as a rank-1 matmul, DMA out.

dma_gather details: indices are int16, so x is viewed as 4 row-chunks
of 25088 and each core's edges are sub-partitioned per (target block,
source chunk).  Per-site capacities are the max count over the 8 cores
(rounded to 128) so one SPMD program fits all cores; slack slots gather
row 0 of the chunk and are zeroed by the selection matrix (trel=-1).
Gather calls are batched per (super-block of SBS blocks, chunk) to
amortize the ~1us SWDGE fixed cost.

Host-side work is restricted to integer graph preprocessing (sorting,
slot layout, degree counts and the scalar norm coefficients derived
from them).  All O(E*D)/O(N*D) tensor math runs on-device.
"""

import os
import sys

import numpy as np

for _p in ("/opt/trn_rl_repo", "/root/.axon_site/_ro/trn_rl_repo"):
    if os.path.isdir(_p) and _p not in sys.path:
        sys.path.append(_p)

from concourse import bacc, bass, mybir
import concourse.tile as tile
from concourse.masks import make_identity

P = 128            # partitions / edge-tile size / node-block size
D = 64             # feature dim (in == out)
N_CORES = 8
N_CHUNKS = 4       # x row-chunks (int16 index limit: chunk rows < 32768)
SBS = 7            # blocks per gather super-block

F32 = mybir.dt.float32
BF16 = mybir.dt.bfloat16
I16 = mybir.dt.int16

_PROGRAM_CACHE: dict = {}


def make_layout(counts_max: np.ndarray, nb: int):
    """Static slot/tile layout shared by all cores.

    counts_max: [nb, N_CHUNKS] max-over-cores edge count per site.
    Returns dict with per-site capacities and tile bases.
    """
    G = -(-counts_max // P)  # [nb, nc] tiles per site
    kb = G.sum(axis=1)  # tiles per block
    n_sb = -(-nb // SBS)
    site_tile_base = np.zeros((nb, N_CHUNKS), np.int64)
    sb_tile_base = np.zeros(n_sb + 1, np.int64)
    t = 0
    for sb in range(n_sb):
        sb_tile_base[sb] = t
        blocks = range(sb * SBS, min((sb + 1) * SBS, nb))
        for c in range(N_CHUNKS):
            for b in blocks:
                site_tile_base[b, c] = t
                t += int(G[b, c])
        sb_tile_base[sb + 1] = t
    return {
        "G": G,
        "kb": kb,
        "nb": nb,
        "n_sb": n_sb,
        "site_tile_base": site_tile_base,
        "sb_tile_base": sb_tile_base,
        "T": int(t),
    }


def build_program(layout, n_pad: int, ch_rows: int, repeat: int = 1, sel_bf16: bool = False):
    nb = layout["nb"]
    G = layout["G"]
    n_sb = layout["n_sb"]
    site_tile_base = layout["site_tile_base"]
    sb_tile_base = layout["sb_tile_base"]
    T = layout["T"]
    sh_rows = nb * P

    nc = bacc.Bacc(
        "TRN2", target_bir_lowering=False, debug=False, num_swdge_queues=4
    )

    xg = nc.dram_tensor("xg", [n_pad, D], F32, kind="ExternalInput")
    xres = nc.dram_tensor("xres", [sh_rows, D], F32, kind="ExternalInput")
    gidx = nc.dram_tensor("gidx", [P, 8 * T], I16, kind="ExternalInput")
    trel = nc.dram_tensor("trel", [P, T], F32, kind="ExternalInput")
    norm = nc.dram_tensor("norm", [P, T], F32, kind="ExternalInput")
    wt = nc.dram_tensor("wt", [D, D], F32, kind="ExternalInput")
    bias = nc.dram_tensor("bias", [1, D], F32, kind="ExternalInput")
    out = nc.dram_tensor("out", [sh_rows, D], F32, kind="ExternalOutput")

    kb_max = int(layout["kb"].max()) if nb else 1
    w_sb_max = max(
        int(sb_tile_base[sb + 1] - sb_tile_base[sb]) for sb in range(n_sb)
    )

    with tile.TileContext(nc) as tc:
        with (
            tc.tile_pool(name="const", bufs=1) as cpool,
            tc.tile_pool(name="gath", bufs=2) as gpool,
            tc.tile_pool(name="msg", bufs=2) as mpool,
            tc.tile_pool(name="sel", bufs=6) as spool,
            tc.tile_pool(name="fin", bufs=3) as fpool,
            tc.tile_pool(name="ps_agg", bufs=4, space="PSUM") as ps_agg,
            tc.tile_pool(name="ps_t", bufs=2, space="PSUM") as ps_t,
            tc.tile_pool(name="ps_o", bufs=2, space="PSUM") as ps_o,
        ):
            iota_i = cpool.tile([P, P], mybir.dt.int32)
            nc.gpsimd.iota(iota_i[:], pattern=[[1, P]], base=0, channel_multiplier=0)
            iota_f = cpool.tile([P, P], F32)
            nc.vector.tensor_copy(iota_f[:], iota_i[:])
            ident = cpool.tile([P, P], F32)
            make_identity(nc, ident[:])
            ones_row = cpool.tile([1, P], F32)
            nc.vector.memset(ones_row[:], 1.0)

            wt_sb = cpool.tile([D, D], F32)
            nc.sync.dma_start(wt_sb[:], wt[:])
            b_sb = cpool.tile([1, D], F32)
            nc.sync.dma_start(b_sb[:], bias[:])
            gidx_sb = cpool.tile([P, 8 * T], I16)
            nc.sync.dma_start(gidx_sb[:], gidx[:])
            trel_f = cpool.tile([P, T], F32)
            nc.sync.dma_start(trel_f[:], trel[:])
            norm_f = cpool.tile([P, T], F32)
            nc.sync.dma_start(norm_f[:], norm[:])
            if sel_bf16:
                trel_sb = cpool.tile([P, T], BF16)
                nc.vector.tensor_copy(trel_sb[:], trel_f[:])
                norm_sb = cpool.tile([P, T], BF16)
                nc.vector.tensor_copy(norm_sb[:], norm_f[:])
                iota_sel = cpool.tile([P, P], BF16)
                nc.vector.tensor_copy(iota_sel[:], iota_f[:])
            else:
                trel_sb, norm_sb, iota_sel = trel_f, norm_f, iota_f

            for _rep in range(repeat):
              for sb in range(n_sb):
                blocks = list(range(sb * SBS, min((sb + 1) * SBS, nb)))
                t0 = int(sb_tile_base[sb])
                w_sb = int(sb_tile_base[sb + 1]) - t0
                gath = gpool.tile([P, w_sb_max, D], F32, tag="gath")
                # one gather call per source chunk, split to <= max_call_tiles
                max_call_tiles = 8  # >=2048 idxs per call desyncs the device
                for c in range(N_CHUNKS):
                    wc = int(sum(G[b, c] for b in blocks))
                    if wc == 0:
                        continue
                    ct0 = int(site_tile_base[blocks[0], c]) - t0
                    for o in range(0, wc, max_call_tiles):
                        w = min(max_call_tiles, wc - o)
                        n_idx = w * P
                        gcol = (t0 + ct0 + o) * 8
                        nc.gpsimd.dma_gather(
                            gath[:, ct0 + o : ct0 + o + w, :],
                            xg[c * ch_rows : (c + 1) * ch_rows, :],
                            gidx_sb[:, gcol : gcol + n_idx // 16],
                            n_idx,
                            n_idx,
                            D,
                            queue_num=c,
                        )
                msg = mpool.tile([P, w_sb_max, D], BF16, tag="msg")
                nc.scalar.copy(msg[:, :w_sb, :], gath[:, :w_sb, :])

                for b in blocks:
                    tiles = []
                    for c in range(N_CHUNKS):
                        for g in range(int(G[b, c])):
                            tiles.append(int(site_tile_base[b, c]) + g)
                    xb = fpool.tile([P, D], F32, tag="xb")
                    nc.sync.dma_start(xb[:], xres[b * P : (b + 1) * P, :])
                    agg = fpool.tile([P, D], F32, tag="agg")
                    if not tiles:
                        nc.vector.tensor_copy(agg[:], xb[:])
                    ps = ps_agg.tile([P, D], F32)
                    for i, t in enumerate(tiles):
                        sel = spool.tile([P, P], BF16)
                        nc.vector.tensor_scalar(
                            sel[:],
                            iota_sel[:],
                            trel_sb[:, t : t + 1],
                            norm_sb[:, t : t + 1],
                            op0=mybir.AluOpType.is_equal,
                            op1=mybir.AluOpType.mult,
                        )
                        nc.tensor.matmul(
                            ps[:],
                            sel[:],
                            msg[:, t - t0, :],
                            start=(i == 0),
                            stop=(i == len(tiles) - 1),
                        )

                    if tiles:
                        nc.vector.tensor_add(agg[:], ps[:], xb[:])

                    ps_tr = ps_t.tile([D, P], F32)
                    nc.tensor.transpose(ps_tr[:], agg[:], ident[:])
                    agg_t = fpool.tile([D, P], F32, tag="agg_t")
                    nc.scalar.copy(agg_t[:], ps_tr[:])

                    po = ps_o.tile([P, D], F32)
                    nc.tensor.matmul(po[:], agg_t[:], wt_sb[:], start=True, stop=False)
                    nc.tensor.matmul(
                        po[:], ones_row[:], b_sb[:], start=False, stop=True
                    )
                    osb = fpool.tile([P, D], F32, tag="osb")
                    nc.scalar.copy(osb[:], po[:])
                    nc.sync.dma_start(out[b * P : (b + 1) * P, :], osb[:])

    nc.compile()
    return nc


def host_prep(x: np.ndarray, edge_index: np.ndarray, n_cores: int, nb: int):
    """Partition edges by (target block, source chunk); build slot arrays.

    Returns (in_maps, layout, n_pad, ch_rows).
    """
    n, d = x.shape
    assert d == D
    total_blocks = n_cores * nb
    n_pad = total_blocks * P
    ch_rows = n_pad // N_CHUNKS
    assert n_pad % N_CHUNKS == 0 and ch_rows <= 32768

    src = np.ascontiguousarray(edge_index[0]).astype(np.int64)
    tgt = np.ascontiguousarray(edge_index[1]).astype(np.int64)
    e = src.shape[0]

    deg = np.bincount(tgt, minlength=n).astype(np.float32)
    dis = 1.0 / np.sqrt(np.maximum(deg, 1.0))
    norm_e = (dis[src] * dis[tgt]).astype(np.float32)

    chunk = src // ch_rows
    order = np.lexsort((chunk, tgt >> 7))
    src_s = src[order]
    tgt_s = tgt[order]
    norm_s = norm_e[order]
    chunk_s = chunk[order]

    blk = tgt_s >> 7  # global block id, 0..total_blocks-1
    seg_id = blk * N_CHUNKS + chunk_s
    seg_counts = np.bincount(seg_id, minlength=total_blocks * N_CHUNKS)
    seg_start = np.zeros(total_blocks * N_CHUNKS + 1, np.int64)
    np.cumsum(seg_counts, out=seg_start[1:])
    rank = np.arange(e, dtype=np.int64) - seg_start[seg_id]

    counts = seg_counts.reshape(n_cores, nb, N_CHUNKS)
    counts_max = counts.max(axis=0)  # [nb, N_CHUNKS]
    layout = make_layout(counts_max, nb)
    T = layout["T"]
    site_slot_base = layout["site_tile_base"] * P  # [nb, N_CHUNKS]

    core_of = blk // nb
    b_local = blk % nb
    slot = site_slot_base[b_local, chunk_s] + rank  # [e] core-local slot

    n_slots = T * P
    flat_trel = np.full((n_cores, n_slots), -1.0, np.float32)
    flat_norm = np.zeros((n_cores, n_slots), np.float32)
    flat_idx = np.zeros((n_cores, n_slots), np.int16)
    flat_trel[core_of, slot] = (tgt_s & 127).astype(np.float32)
    flat_norm[core_of, slot] = norm_s
    flat_idx[core_of, slot] = (src_s - chunk_s * ch_rows).astype(np.int16)

    xg = np.zeros((n_pad, D), np.float32)
    xg[:n] = x

    # wrap gather indices per call region: [sb][c] contiguous slot ranges
    sb_slot_base = layout["sb_tile_base"] * P
    G = layout["G"]
    n_sb = layout["n_sb"]
    gidx_all = np.empty((n_cores, P, 8 * T), np.int16)
    for sb in range(n_sb):
        blocks = range(sb * SBS, min((sb + 1) * SBS, nb))
        for c in range(N_CHUNKS):
            wc = int(sum(G[b, c] for b in blocks))
            if wc == 0:
                continue
            s0 = int(site_slot_base[blocks[0], c])
            L = wc * P
            seg = flat_idx[:, s0 : s0 + L]  # [cores, L]
            wrapped = seg.reshape(n_cores, L // 16, 16).transpose(0, 2, 1)
            gidx_all[:, :, s0 // 16 : (s0 + L) // 16] = np.tile(wrapped, (1, 8, 1))

    trel_all = flat_trel.reshape(n_cores, T, P).transpose(0, 2, 1)
    norm_all = flat_norm.reshape(n_cores, T, P).transpose(0, 2, 1)

    in_maps = []
    for c in range(n_cores):
        in_maps.append(
            {
                "xg": xg,
                "xres": np.ascontiguousarray(xg[c * nb * P : (c + 1) * nb * P]),
                "gidx": np.ascontiguousarray(gidx_all[c]),
                "trel": np.ascontiguousarray(trel_all[c]),
                "norm": np.ascontiguousarray(norm_all[c]),
            }
        )
    return in_maps, layout, n_pad, ch_rows


def kernel(x: np.ndarray, edge_index: np.ndarray, W: np.ndarray, b: np.ndarray) -> np.ndarray:
    from concourse.bass_utils import run_bass_kernel_spmd

    x = np.ascontiguousarray(x, dtype=np.float32)
    W = np.ascontiguousarray(W, dtype=np.float32)
    b = np.ascontiguousarray(b, dtype=np.float32)
    n = x.shape[0]

    rows_per_core = -(-n // N_CORES)
    nb = -(-rows_per_core // P)  # node blocks per core
    in_maps, layout, n_pad, ch_rows = host_prep(x, edge_index, N_CORES, nb)
    wt = np.ascontiguousarray(W.T)
    brow = np.ascontiguousarray(b[None, :])
    for m in in_maps:
        m["wt"] = wt
        m["bias"] = brow

    key = (nb, n_pad, layout["G"].tobytes())
    nc = _PROGRAM_CACHE.get(key)
    if nc is None:
        nc = build_program(layout, n_pad, ch_rows)
        _PROGRAM_CACHE[key] = nc

    res = run_bass_kernel_spmd(nc, in_maps, core_ids=list(range(N_CORES)))
    shards = [res.results[c]["out"] for c in range(N_CORES)]
    return np.concatenate(shards, axis=0)[:n].astype(np.float32)
